# revision 10
# baseline (speedup 1.0000x reference)
"""BitLinear MLP Trainium2 kernel (8 NeuronCores, data-parallel over tokens).

Reference computation (see problem):
    residual = x
    xn  = rmsnorm(x) * norm_weight
    wq_up, s_up     = ternary_quantize(weight_up)    # global absmean scale
    wq_down, s_down = ternary_quantize(weight_down)
    x_q, s_x = activation_quant(xn)                  # per-token absmax int8
    h   = (x_q * s_x) @ (wq_up * s_up).T
    h   = silu(h)
    out = h @ (wq_down * s_down).T
    return residual + out * gamma

Key numeric facts used:
  * x_q holds integers in [-127, 127] and wq holds {-1, 0, +1} -> both exact
    in bf16; the up-projection matmul is therefore EXACT in bf16 with fp32
    PSUM accumulation (|sum| < 2^24).  Scales are applied after the matmul.
  * round-half-to-even == (v + 2^23) - 2^23 in fp32 for |v| << 2^23.

Sharding: data-parallel — each core takes 2048 of the 16384 tokens and the
full weights.  The global absmean scales are computed from per-core weight
slices + a tiny AllReduce.  No other collectives.
"""

import numpy as np

import concourse.bass as bass
import concourse.mybir as mybir
import concourse.tile as tile
from concourse import bacc
from concourse.bass_utils import run_bass_kernel_spmd
from concourse.masks import make_identity

F32 = mybir.dt.float32
BF16 = mybir.dt.bfloat16
AX = mybir.AxisListType
OP = mybir.AluOpType
ACT = mybir.ActivationFunctionType

EPS_NORM = 1e-6
EPS_Q = 1e-8
QB = 127.0
R = 2.0**23  # round-to-nearest-even trick constant


def full_cfg():
    return dict(
        n_cores=8,
        B=4, S=4096,
        dim=2048, hid=8192,
        sb=512,        # tokens per subblock
        hgroup=256,    # mm1 h-group (stationary tile free width)
        doutg=512,     # mm2 dout group
        ch_up=1024,    # W_up ternarize/cache chunk (rows of [hid, dim])
        ch_dn=1024,    # W_down ternarize/cache chunk (rows of [dim, hid])
        slice_d=1024,  # ternarize free-dim slice for W_up rows
        slice_h=1024,  # ternarize free-dim slice for W_down rows
    )


def mini_cfg():
    return dict(
        n_cores=8,
        B=1, S=2048,
        dim=256, hid=512,
        sb=128,
        hgroup=256,
        doutg=256,
        ch_up=512,
        ch_dn=256,
        slice_d=256,
        slice_h=512,
    )


def build_program(cfg):
    """Build the per-core Bass/Tile program (SPMD across n_cores)."""
    n_cores = cfg["n_cores"]
    dim, hid = cfg["dim"], cfg["hid"]
    ntok = cfg["B"] * cfg["S"]
    assert ntok % n_cores == 0
    tpc = ntok // n_cores             # tokens per core
    sb = cfg["sb"]
    assert tpc % sb == 0 and sb % 128 == 0
    n_sb = tpc // sb
    tokt = sb // 128                  # 128-token tiles per subblock
    ndb = dim // 128                  # d-blocks (contraction tiles, mm1)
    nht = hid // 128                  # h-tiles (contraction tiles, mm2)
    hgroup = cfg["hgroup"]
    nhg = hid // hgroup
    hsub = hgroup // 128
    doutg = cfg["doutg"]
    ng = dim // doutg
    ch_up, ch_dn = cfg["ch_up"], cfg["ch_dn"]
    slice_d, slice_h = cfg["slice_d"], cfg["slice_h"]
    assert hid % ch_up == 0 and dim % ch_dn == 0
    nch_up, nch_dn = hid // ch_up, dim // ch_dn
    assert dim % slice_d == 0 and hid % slice_h == 0
    n_w = hid * dim                   # element count of each weight matrix
    # per-core slices for the global weight-mean
    up_sl_rows = hid // n_cores
    dn_sl_rows = dim // n_cores

    nc = bacc.Bacc(
        "TRN2", target_bir_lowering=False, debug=False, num_devices=n_cores
    )

    # ---- I/O ----------------------------------------------------------
    xs = nc.dram_tensor("xs", [tpc, dim], F32, kind="ExternalInput").ap()
    wup = nc.dram_tensor("wup", [hid, dim], F32, kind="ExternalInput").ap()
    wdn = nc.dram_tensor("wdn", [dim, hid], F32, kind="ExternalInput").ap()
    nw = nc.dram_tensor("nw", [dim], F32, kind="ExternalInput").ap()
    gm = nc.dram_tensor("gm", [dim], F32, kind="ExternalInput").ap()
    wup_sl = nc.dram_tensor("wup_sl", [up_sl_rows, dim], F32,
                            kind="ExternalInput").ap()
    wdn_sl = nc.dram_tensor("wdn_sl", [dn_sl_rows, hid], F32,
                            kind="ExternalInput").ap()
    ys = nc.dram_tensor("ys", [tpc, dim], F32, kind="ExternalOutput").ap()

    with tile.TileContext(nc) as tc:
        _emit(tc, cfg, locals())
    nc.compile()
    return nc


def _emit(tc, cfg, v):
    nc = tc.nc
    dim, hid = cfg["dim"], cfg["hid"]
    sb, hgroup, doutg = cfg["sb"], cfg["hgroup"], cfg["doutg"]
    n_sb, tokt, ndb, nht = v["n_sb"], v["tokt"], v["ndb"], v["nht"]
    nhg, hsub, ng = v["nhg"], v["hsub"], v["ng"]
    ch_up, ch_dn = cfg["ch_up"], cfg["ch_dn"]
    slice_d, slice_h = cfg["slice_d"], cfg["slice_h"]
    nch_up, nch_dn = v["nch_up"], v["nch_dn"]
    n_w = v["n_w"]
    tpc = v["tpc"]
    xs, wup, wdn, nw, gm = v["xs"], v["wup"], v["wdn"], v["nw"], v["gm"]
    wup_sl, wdn_sl, ys = v["wup_sl"], v["wdn_sl"], v["ys"]
    n_cores = cfg["n_cores"]

    lvl = cfg.get("stages", 5)
    import contextlib
    ctx = contextlib.ExitStack()
    with ctx:
        consts = ctx.enter_context(tc.tile_pool(name="consts", bufs=1))
        small = ctx.enter_context(tc.tile_pool(name="small", bufs=2))
        wstage = ctx.enter_context(tc.tile_pool(name="wstage", bufs=2))
        xpool = ctx.enter_context(tc.tile_pool(name="xpool", bufs=2))
        xtp = ctx.enter_context(tc.tile_pool(name="xtp", bufs=1))
        htp = ctx.enter_context(tc.tile_pool(name="htp", bufs=1))
        wtp = ctx.enter_context(tc.tile_pool(name="wtp", bufs=2))
        wdtp = ctx.enter_context(tc.tile_pool(name="wdtp", bufs=4))
        opool = ctx.enter_context(tc.tile_pool(name="opool", bufs=2))
        ps1 = ctx.enter_context(tc.tile_pool(name="ps1", bufs=2, space="PSUM"))
        ps2 = ctx.enter_context(tc.tile_pool(name="ps2", bufs=1, space="PSUM"))
        psx = ctx.enter_context(tc.tile_pool(name="psx", bufs=2, space="PSUM"))
        dram = ctx.enter_context(tc.tile_pool(name="dram", bufs=1, space="DRAM"))

        # ---- constants ------------------------------------------------
        ident = consts.tile([128, 128], BF16)
        make_identity(nc, ident)
        eps_b = consts.tile([128, 1], F32)
        nc.vector.memset(eps_b, EPS_NORM)
        ones_col = consts.tile([128, 1], F32)
        nc.vector.memset(ones_col, 1.0)
        nw_b = consts.tile([128, dim], F32)
        nc.gpsimd.dma_start(out=nw_b, in_=nw[None].to_broadcast((128, dim)))
        ge = consts.tile([128, dim], F32)
        nc.gpsimd.dma_start(out=ge, in_=gm[None].to_broadcast((128, dim)))

        # ---- phase 0: global absmean scales ---------------------------
        # partial |w| sums over this core's slices, fp32
        nsum_u = (v["up_sl_rows"] + 127) // 128
        nsum_d = (v["dn_sl_rows"] + 127) // 128
        sums = small.tile([128, 2], F32)
        nc.vector.memset(sums, 0.0)
        partu = small.tile([128, max(nsum_u * (dim // slice_d), 2)], F32)
        nc.vector.memset(partu, 0.0)
        pi = 0
        for r in range(0, v["up_sl_rows"], 128):
            rr = min(128, v["up_sl_rows"] - r)
            for f in range(0, dim, slice_d):
                wt = wstage.tile([128, slice_d], F32, tag="wt")
                nc.gpsimd.dma_start(out=wt[:rr], in_=wup_sl[r:r + rr,
                                                           f:f + slice_d])
                nc.vector.tensor_reduce(
                    out=partu[:rr, pi:pi + 1], in_=wt[:rr], axis=AX.X,
                    op=OP.add, apply_absolute_value=True)
                pi += 1
        nc.vector.tensor_reduce(out=sums[:, 0:1], in_=partu, axis=AX.X,
                                op=OP.add)
        partd = small.tile([128, max(nsum_d * (hid // slice_h), 2)], F32)
        nc.vector.memset(partd, 0.0)
        pi = 0
        for r in range(0, v["dn_sl_rows"], 128):
            rr = min(128, v["dn_sl_rows"] - r)
            for f in range(0, hid, slice_h):
                wt = wstage.tile([128, slice_h], F32, tag="wt")
                nc.gpsimd.dma_start(out=wt[:rr], in_=wdn_sl[r:r + rr,
                                                            f:f + slice_h])
                nc.vector.tensor_reduce(
                    out=partd[:rr, pi:pi + 1], in_=wt[:rr], axis=AX.X,
                    op=OP.add, apply_absolute_value=True)
                pi += 1
        nc.vector.tensor_reduce(out=sums[:, 1:2], in_=partd, axis=AX.X,
                                op=OP.add)

        # cross-partition sum via PE: [2,1] = sums.T @ ones
        ps_s = ps1.tile([2, 1], F32, tag="mm1")
        nc.tensor.matmul(ps_s, lhsT=sums, rhs=ones_col, start=True, stop=True)
        sums_sb = small.tile([2, 1], F32)
        nc.vector.tensor_copy(out=sums_sb, in_=ps_s)

        cc_in = dram.tile([2], F32)
        cc_out = dram.tile([2], F32)
        nc.gpsimd.dma_start(out=cc_in, in_=sums_sb)
        nc.gpsimd.collective_compute(
            "AllReduce", OP.add,
            replica_groups=[list(range(n_cores))],
            ins=[cc_in[:]], outs=[cc_out[:]],
        )
        # broadcast totals to all partitions, then finish scalar math
        tot_b = consts.tile([128, 2], F32)
        nc.gpsimd.dma_start(out=tot_b, in_=cc_out[None].to_broadcast((128, 2)))
        s2 = consts.tile([128, 2], F32)   # [s_up, s_down] per partition
        nc.vector.tensor_scalar(out=s2, in0=tot_b, scalar1=1.0 / n_w,
                                scalar2=EPS_Q, op0=OP.mult, op1=OP.max)
        inv2 = consts.tile([128, 2], F32)  # [1/s_up, 1/s_down]
        nc.vector.reciprocal(out=inv2, in_=s2)
        su127_b = consts.tile([128, 1], F32)  # s_up / 127
        nc.vector.tensor_scalar(out=su127_b, in0=s2[:, 0:1],
                                scalar1=1.0 / QB, scalar2=None, op0=OP.mult)
        # gamma_eff = gamma * s_down  (applied to integer down-proj output)
        nc.vector.tensor_scalar(out=ge, in0=ge, scalar1=s2[:, 1:2],
                                scalar2=None, op0=OP.mult)

        # ---- ternarize weights into bf16 DRAM caches ------------------
        # wq = clip(round(w / s), -1, 1), natural layout, chunked so the
        # matmul phase can start before the whole matrix is done.
        def ternarize(dst_chunks, src, rows, n_rows_chunk, fslice, fdim, inv_sl):
            for c in range(len(dst_chunks)):
                for r in range(0, n_rows_chunk, 128):
                    row0 = c * n_rows_chunk + r
                    for f in range(0, fdim, fslice):
                        wt = wstage.tile([128, fslice], F32, tag="wt")
                        nc.gpsimd.dma_start(
                            out=wt, in_=src[row0:row0 + 128, f:f + fslice])
                        nc.vector.tensor_scalar(
                            out=wt, in0=wt, scalar1=inv_sl, scalar2=R,
                            op0=OP.mult, op1=OP.add)
                        nc.vector.tensor_scalar(
                            out=wt, in0=wt, scalar1=-R, scalar2=None,
                            op0=OP.add)
                        wq = wstage.tile([128, fslice], BF16, tag="wq")
                        nc.vector.tensor_scalar(
                            out=wq, in0=wt, scalar1=1.0, scalar2=-1.0,
                            op0=OP.min, op1=OP.max)
                        nc.gpsimd.dma_start(
                            out=dst_chunks[c][r:r + 128, f:f + fslice], in_=wq)

        wupq = [dram.tile([ch_up, dim], BF16, tag=f"wupq{c}", name=f"wupq{c}")
                for c in range(nch_up)]
        wdnq = [dram.tile([ch_dn, hid], BF16, tag=f"wdnq{c}", name=f"wdnq{c}")
                for c in range(nch_dn)]
        if lvl >= 1:
            ternarize(wupq, wup, hid, ch_up, slice_d, dim, inv2[:, 0:1])
            ternarize(wdnq, wdn, dim, ch_dn, slice_h, hid, inv2[:, 1:2])

        s_dram = dram.tile([tpc], F32)

        # ---- main loop over token subblocks ---------------------------
        if lvl < 5:
            dummy = opool.tile([128, dim], F32, tag="dummy")
            nc.vector.memset(dummy, 0.0)
            for r in range(0, tpc, 128):
                nc.gpsimd.dma_start(out=ys[r:r + 128, :], in_=dummy)
        for isb in range(n_sb if lvl >= 2 else 0):
            t0 = isb * sb

            # -- x-prep: rmsnorm + activation quant (per 128-token tile)
            xq_tiles = []
            for tt in range(tokt):
                row0 = t0 + tt * 128
                xt = xpool.tile([128, dim], F32, tag="xt")
                nc.gpsimd.dma_start(out=xt, in_=xs[row0:row0 + 128, :])
                xw = xpool.tile([128, dim], F32, tag="xw")
                ssq = small.tile([128, 1], F32, tag="ssq")
                # x*x (scratch into xw) then row sum -> ssq
                nc.vector.tensor_tensor(out=xw, in0=xt, in1=xt, op=OP.mult)
                nc.vector.tensor_reduce(out=ssq, in_=xw, axis=AX.X, op=OP.add)
                am0 = small.tile([128, 1], F32, tag="am0")
                # xw = x * norm_weight, absmax -> am0
                nc.vector.tensor_tensor(out=xw, in0=xt, in1=nw_b, op=OP.mult)
                nc.vector.tensor_reduce(out=am0, in_=xw, axis=AX.X, op=OP.max,
                                        apply_absolute_value=True)
                sig = small.tile([128, 1], F32, tag="sig")
                nc.scalar.activation(out=sig, in_=ssq, func=ACT.Sqrt,
                                     bias=eps_b, scale=1.0 / dim)
                rstd = small.tile([128, 1], F32, tag="rstd")
                nc.vector.reciprocal(out=rstd, in_=sig)
                gt = small.tile([128, 1], F32, tag="gt")
                nc.vector.tensor_scalar(out=gt, in0=am0, scalar1=rstd,
                                        scalar2=EPS_Q, op0=OP.mult, op1=OP.max)
                invg = small.tile([128, 1], F32, tag="invg")
                nc.vector.reciprocal(out=invg, in_=gt)
                rc = small.tile([128, 1], F32, tag="rc")
                nc.vector.tensor_scalar(out=rc, in0=invg, scalar1=rstd,
                                        scalar2=QB, op0=OP.mult, op1=OP.mult)
                # x_q = round(xw * rc) via +-2^23, exact ints in bf16
                nc.vector.tensor_scalar(out=xw, in0=xw, scalar1=rc, scalar2=R,
                                        op0=OP.mult, op1=OP.add)
                xq = xpool.tile([128, dim], BF16, tag="xq", bufs=tokt + 1)
                nc.vector.tensor_scalar(out=xq, in0=xw, scalar1=-R,
                                        scalar2=None, op0=OP.add)
                xq_tiles.append(xq)
                # stash per-token scale (gamma_tok) for later broadcast
                nc.gpsimd.dma_start(out=s_dram[row0:row0 + 128], in_=gt)

            if lvl < 3:
                continue
            # -- transpose x_q -> Xt[dj] = [128d, sb tokens] (PE transpose)
            xt_tiles = []
            for dj in range(ndb):
                pxp = psx.tile([128, sb], BF16, tag="xp")
                for tt in range(tokt):
                    nc.tensor.transpose(
                        pxp[:, tt * 128:(tt + 1) * 128],
                        xq_tiles[tt][:, dj * 128:(dj + 1) * 128], ident)
                xtt = xtp.tile([128, sb], BF16, tag=f"xt{dj}")
                nc.vector.tensor_copy(out=xtt, in_=pxp)
                xt_tiles.append(xtt)

            # -- per-token effective scale row, broadcast to 128 partitions
            s_eff = xpool.tile([128, sb], F32, tag="seff", bufs=2)
            nc.gpsimd.dma_start(
                out=s_eff, in_=s_dram[t0:t0 + sb][None].to_broadcast((128, sb)))
            nc.vector.tensor_scalar(out=s_eff, in0=s_eff, scalar1=su127_b,
                                    scalar2=None, op0=OP.mult)

            if lvl < 4:
                continue
            # -- mm1: H^T[h, tok] = Wq_up^T.T @ Xt ; scale + silu -> bf16
            ht_tiles = [None] * nht
            for hg in range(nhg):
                wT = wtp.tile([128, ndb, hgroup], BF16, tag="wT")
                c = (hg * hgroup) // ch_up
                hr0 = hg * hgroup - c * ch_up
                for dj in range(ndb):
                    nc.sync.dma_start(
                        out=wT[:, dj, :],
                        in_=wupq[c][hr0:hr0 + hgroup, dj * 128:(dj + 1) * 128],
                        transpose=True)
                for hs in range(hsub):
                    hti = hg * hsub + hs
                    ph = ps1.tile([128, sb], F32, tag="mm1")
                    for dj in range(ndb):
                        nc.tensor.matmul(
                            ph, lhsT=wT[:, dj, hs * 128:(hs + 1) * 128],
                            rhs=xt_tiles[dj], start=(dj == 0),
                            stop=(dj == ndb - 1))
                    hsf = opool.tile([128, sb], F32, tag="hsf")
                    nc.vector.tensor_tensor(out=hsf, in0=ph, in1=s_eff,
                                            op=OP.mult)
                    htt = htp.tile([128, sb], BF16, tag=f"ht{hti}")
                    nc.scalar.activation(out=htt, in_=hsf, func=ACT.Silu)
                    ht_tiles[hti] = htt

            if lvl < 5:
                continue
            # -- mm2: out[tok, dout] += Ht.T @ Wq_dn^T ; scale + residual
            for g in range(ng):
                d0 = g * doutg
                c = d0 // ch_dn
                dr0 = d0 - c * ch_dn
                pos = [ps2.tile([128, doutg], F32, tag=f"mm2_{t}",
                                name=f"mm2_{t}")
                       for t in range(tokt)]
                for h in range(nht):
                    wdT = wdtp.tile([128, doutg], BF16, tag="wdT")
                    nc.sync.dma_start(
                        out=wdT,
                        in_=wdnq[c][dr0:dr0 + doutg, h * 128:(h + 1) * 128],
                        transpose=True)
                    for t in range(tokt):
                        nc.tensor.matmul(
                            pos[t], lhsT=ht_tiles[h][:, t * 128:(t + 1) * 128],
                            rhs=wdT, start=(h == 0), stop=(h == nht - 1))
                for t in range(tokt):
                    row0 = t0 + t * 128
                    osl = opool.tile([128, doutg], F32, tag="osl")
                    nc.vector.tensor_tensor(out=osl, in0=pos[t],
                                            in1=ge[:, d0:d0 + doutg],
                                            op=OP.mult)
                    xr = opool.tile([128, doutg], F32, tag="xr")
                    nc.gpsimd.dma_start(
                        out=xr, in_=xs[row0:row0 + 128, d0:d0 + doutg])
                    nc.vector.tensor_tensor(out=osl, in0=osl, in1=xr,
                                            op=OP.add)
                    nc.gpsimd.dma_start(
                        out=ys[row0:row0 + 128, d0:d0 + doutg], in_=osl)


_PROGRAM_CACHE = {}


def _get_program(key, cfg):
    if key not in _PROGRAM_CACHE:
        _PROGRAM_CACHE[key] = build_program(cfg)
    return _PROGRAM_CACHE[key]


def run(cfg, x, weight_up, weight_down, norm_weight, gamma, **run_kwargs):
    n_cores = cfg["n_cores"]
    dim, hid = cfg["dim"], cfg["hid"]
    ntok = cfg["B"] * cfg["S"]
    tpc = ntok // n_cores

    nc = _get_program(("cfg", cfg["dim"], cfg["hid"], cfg["B"], cfg["S"],
                       cfg["sb"]), cfg)

    x2 = np.ascontiguousarray(x.reshape(ntok, dim).astype(np.float32))
    wu = np.ascontiguousarray(weight_up.astype(np.float32))
    wd = np.ascontiguousarray(weight_down.astype(np.float32))
    nwv = np.ascontiguousarray(norm_weight.astype(np.float32))
    gmv = np.ascontiguousarray(gamma.astype(np.float32))

    usr = hid // n_cores
    dsr = dim // n_cores
    in_maps = []
    for c in range(n_cores):
        in_maps.append({
            "xs": x2[c * tpc:(c + 1) * tpc],
            "wup": wu,
            "wdn": wd,
            "nw": nwv,
            "gm": gmv,
            "wup_sl": wu[c * usr:(c + 1) * usr],
            "wdn_sl": wd[c * dsr:(c + 1) * dsr],
        })

    res = run_bass_kernel_spmd(nc, in_maps, core_ids=list(range(n_cores)),
                               **run_kwargs)
    out = np.concatenate([res.results[c]["ys"] for c in range(n_cores)], axis=0)
    out = out.reshape(cfg["B"], cfg["S"], dim)
    return out, res


def kernel(x, weight_up, weight_down, norm_weight, gamma):
    out, _ = run(full_cfg(), x, weight_up, weight_down, norm_weight, gamma)
    return out.astype(np.float32)


if __name__ == "__main__":
    cfg = full_cfg()
    nc = build_program(cfg)
    print("build OK")


# revision 12
# speedup vs baseline: 1.0773x; 1.0773x over previous
"""BitLinear MLP on 8 trn2 cores — TP(4) x DP(2) hybrid.

Per core (group g = core//4, rank r = core%4):
  * weights: rank's hidden-slice (hid/4 = 2048 rows of W_up, cols of W_down)
    ternarized on device and held RESIDENT in SBUF in transposed bf16 layout
    (wupT [d,h_loc], wdnT [h_loc,d]) — loaded once, no per-token-block
    weight streaming.
  * tokens: group owns 8192 tokens; each rank quantizes/transposes its own
    2048, then chunk-wise AllGather (bf16 x_q^T + scales) shares them across
    the TP group.
  * mm1: H^T[h_loc, tok] = wupT.T @ Xt, scale+silu in place; mm2 partial
    out[tok, d] accumulated over the local h-slice only.
  * partial outs (bf16) ReduceScatter-added across the TP group; each rank
    gets back exactly its own 2048 tokens, applies gamma*s_down + residual.
bf16 partials are safe: the MLP branch is scaled by gamma=1e-5 before the
residual add, so 0.4% bf16 noise lands ~1e-7 relative on the output.
"""

import numpy as np

import concourse.bass as bass
import concourse.mybir as mybir
import concourse.tile as tile
from concourse import bacc
from concourse.bass_utils import run_bass_kernel_spmd
from concourse.masks import make_identity

F32 = mybir.dt.float32
BF16 = mybir.dt.bfloat16
AX = mybir.AxisListType
OP = mybir.AluOpType
ACT = mybir.ActivationFunctionType

EPS_NORM = 1e-6
EPS_Q = 1e-8
QB = 127.0
R = 2.0**23


def tp_full_cfg():
    return dict(
        n_cores=8, tp=4,
        B=4, S=4096,
        dim=2048, hid=8192,
        sb=512,          # tokens per mm subblock (= own tokens per chunk)
        n_chunks=4,      # AG/RS granularity per rank
        slice_w=1024,    # ternarize free-dim slice
    )


def tp_mini_cfg():
    return dict(
        n_cores=8, tp=4,
        B=1, S=2048,
        dim=256, hid=512,
        sb=128,
        n_chunks=2,
        slice_w=256,
    )


def build_program(cfg):
    n_cores, tp = cfg["n_cores"], cfg["tp"]
    dp = n_cores // tp
    dim, hid = cfg["dim"], cfg["hid"]
    ntok = cfg["B"] * cfg["S"]
    grp_tok = ntok // dp              # tokens per TP group
    own = grp_tok // tp               # tokens prepped/owned per core
    sb = cfg["sb"]
    n_chunks = cfg["n_chunks"]
    assert own == sb * n_chunks
    tokt = sb // 128
    ndb = dim // 128                  # d-blocks
    h_loc = hid // tp                 # local hidden slice
    nht = h_loc // 128                # local h-tiles
    doutg = min(512, dim)
    ng = dim // doutg                 # mm2 dout groups
    slice_w = cfg["slice_w"]
    n_w = hid * dim
    up_sl_rows = hid // n_cores
    dn_sl_rows = dim // n_cores

    nc = bacc.Bacc(
        "TRN2", target_bir_lowering=False, debug=False, num_devices=n_cores
    )

    xs = nc.dram_tensor("xs", [own, dim], F32, kind="ExternalInput").ap()
    wup_tp = nc.dram_tensor("wup_tp", [h_loc, dim], F32,
                            kind="ExternalInput").ap()
    wdn_tp = nc.dram_tensor("wdn_tp", [dim, h_loc], F32,
                            kind="ExternalInput").ap()
    nw = nc.dram_tensor("nw", [dim], F32, kind="ExternalInput").ap()
    gm = nc.dram_tensor("gm", [dim], F32, kind="ExternalInput").ap()
    wup_sl = nc.dram_tensor("wup_sl", [up_sl_rows, dim], F32,
                            kind="ExternalInput").ap()
    wdn_sl = nc.dram_tensor("wdn_sl", [dn_sl_rows, hid], F32,
                            kind="ExternalInput").ap()
    ys = nc.dram_tensor("ys", [own, dim], F32, kind="ExternalOutput").ap()

    with tile.TileContext(nc) as tc:
        _emit_tp(tc, cfg, locals())
    nc.compile()
    return nc


def _emit_tp(tc, cfg, v):
    nc = tc.nc
    n_cores, tp = cfg["n_cores"], cfg["tp"]
    dp = n_cores // tp
    dim, hid = cfg["dim"], cfg["hid"]
    sb, n_chunks = cfg["sb"], cfg["n_chunks"]
    tokt, ndb, nht, ng = v["tokt"], v["ndb"], v["nht"], v["ng"]
    doutg = v["doutg"]
    h_loc, own, n_w = v["h_loc"], v["own"], v["n_w"]
    slice_w = cfg["slice_w"]
    xs, wup_tp, wdn_tp, nw, gm = (v["xs"], v["wup_tp"], v["wdn_tp"],
                                  v["nw"], v["gm"])
    wup_sl, wdn_sl, ys = v["wup_sl"], v["wdn_sl"], v["ys"]
    groups = [list(range(g * tp, (g + 1) * tp)) for g in range(dp)]

    import contextlib
    ctx = contextlib.ExitStack()
    with ctx:
        consts = ctx.enter_context(tc.tile_pool(name="consts", bufs=1))
        small = ctx.enter_context(tc.tile_pool(name="small", bufs=2))
        wres = ctx.enter_context(tc.tile_pool(name="wres", bufs=1))
        wstage = ctx.enter_context(tc.tile_pool(name="wstage", bufs=2))
        xpool = ctx.enter_context(tc.tile_pool(name="xpool", bufs=1))
        xtp = ctx.enter_context(tc.tile_pool(name="xtp", bufs=2))
        htp = ctx.enter_context(tc.tile_pool(name="htp", bufs=1))
        opool = ctx.enter_context(tc.tile_pool(name="opool", bufs=2))
        ps1 = ctx.enter_context(tc.tile_pool(name="ps1", bufs=3, space="PSUM"))
        ps2 = ctx.enter_context(tc.tile_pool(name="ps2", bufs=1, space="PSUM"))
        psx = ctx.enter_context(tc.tile_pool(name="psx", bufs=1, space="PSUM"))
        dram = ctx.enter_context(tc.tile_pool(name="dram", bufs=1,
                                              space="DRAM"))

        # ---- constants ---------------------------------------------------
        ident = consts.tile([128, 128], BF16)
        make_identity(nc, ident)
        eps_b = consts.tile([128, 1], F32)
        nc.vector.memset(eps_b, EPS_NORM)
        ones_col = consts.tile([128, 1], F32)
        nc.vector.memset(ones_col, 1.0)
        nw_b = consts.tile([128, dim], BF16)
        nc.gpsimd.dma_start(out=nw_b, in_=nw[None].to_broadcast((128, dim)))
        ge = consts.tile([128, dim], F32)
        nc.gpsimd.dma_start(out=ge, in_=gm[None].to_broadcast((128, dim)))

        # ---- phase 0: global absmean scales (sharded + AllReduce) -------
        sums = small.tile([128, 2], F32)
        nc.vector.memset(sums, 0.0)
        nparts = max((v["up_sl_rows"] + 127) // 128 * (dim // slice_w),
                     (v["dn_sl_rows"] + 127) // 128 * (hid // slice_w), 2)
        part = small.tile([128, 2, nparts], F32)
        nc.vector.memset(part, 0.0)
        for col, (src, rows, fdim) in enumerate(
                [(wup_sl, v["up_sl_rows"], dim),
                 (wdn_sl, v["dn_sl_rows"], hid)]):
            pi = 0
            for r0 in range(0, rows, 128):
                rr = min(128, rows - r0)
                for f in range(0, fdim, slice_w):
                    wt = wstage.tile([128, slice_w], F32, tag="wt")
                    nc.gpsimd.dma_start(out=wt[:rr],
                                        in_=src[r0:r0 + rr, f:f + slice_w])
                    nc.vector.tensor_reduce(
                        out=part[:rr, col, pi:pi + 1], in_=wt[:rr], axis=AX.X,
                        op=OP.add, apply_absolute_value=True)
                    pi += 1
            nc.vector.tensor_reduce(out=sums[:, col:col + 1],
                                    in_=part[:, col, :], axis=AX.X, op=OP.add)
        ps_s = ps1.tile([2, 1], F32, tag="mm1")
        nc.tensor.matmul(ps_s, lhsT=sums, rhs=ones_col, start=True, stop=True)
        sums_sb = small.tile([2, 1], F32)
        nc.vector.tensor_copy(out=sums_sb, in_=ps_s)
        cc_in = dram.tile([2], F32)
        cc_out = dram.tile([2], F32)
        nc.gpsimd.dma_start(out=cc_in, in_=sums_sb)
        nc.gpsimd.collective_compute(
            "AllReduce", OP.add, replica_groups=[list(range(n_cores))],
            ins=[cc_in[:]], outs=[cc_out[:]])
        tot_b = consts.tile([128, 2], F32)
        nc.gpsimd.dma_start(out=tot_b, in_=cc_out[None].to_broadcast((128, 2)))
        s2 = consts.tile([128, 2], F32)
        nc.vector.tensor_scalar(out=s2, in0=tot_b, scalar1=1.0 / n_w,
                                scalar2=EPS_Q, op0=OP.mult, op1=OP.max)
        inv2 = consts.tile([128, 2], F32)
        nc.vector.reciprocal(out=inv2, in_=s2)
        su127_b = consts.tile([128, 1], F32)
        nc.vector.tensor_scalar(out=su127_b, in0=s2[:, 0:1], scalar1=1.0 / QB,
                                scalar2=None, op0=OP.mult)
        # gamma_eff = gamma * s_down
        nc.vector.tensor_scalar(out=ge, in0=ge, scalar1=s2[:, 1:2],
                                scalar2=None, op0=OP.mult)

        # ---- ternarize local weight slices -> bf16 natural DRAM ---------
        def ternarize(dst, src, rows, fdim, inv_sl):
            sl_w = min(slice_w, fdim)
            for r0 in range(0, rows, 128):
                for f in range(0, fdim, sl_w):
                    wt = wstage.tile([128, sl_w], F32, tag="wt",
                                     name="wt")
                    nc.gpsimd.dma_start(out=wt,
                                        in_=src[r0:r0 + 128, f:f + sl_w])
                    nc.vector.tensor_scalar(out=wt, in0=wt, scalar1=inv_sl,
                                            scalar2=R, op0=OP.mult, op1=OP.add)
                    nc.vector.tensor_scalar(out=wt, in0=wt, scalar1=-R,
                                            scalar2=None, op0=OP.add)
                    wq = wstage.tile([128, sl_w], BF16, tag="wq",
                                     name="wq")
                    nc.vector.tensor_scalar(out=wq, in0=wt, scalar1=1.0,
                                            scalar2=-1.0, op0=OP.min,
                                            op1=OP.max)
                    nc.gpsimd.dma_start(out=dst[r0:r0 + 128, f:f + sl_w],
                                        in_=wq)

        wupq = dram.tile([h_loc, dim], BF16)
        wdnq = dram.tile([dim, h_loc], BF16)
        ternarize(wupq, wup_tp, h_loc, dim, inv2[:, 0:1])
        ternarize(wdnq, wdn_tp, dim, h_loc, inv2[:, 1:2])

        # ---- resident transposed weights in SBUF ------------------------
        # wupT[dj] = [128 d, h_loc],  wdnT[hj] = [128 h, dim]
        wupT = []
        for dj in range(ndb):
            wu_t = wres.tile([128, h_loc], BF16, tag=f"wu{dj}",
                             name=f"wu{dj}")
            nc.sync.dma_start(out=wu_t, in_=wupq[:, dj * 128:(dj + 1) * 128],
                              transpose=True)
            wupT.append(wu_t)
        # wdnT_dram[h_loc, dim]: transposed bf16 cache of W_down slice
        wdnT_dram = dram.tile([h_loc, dim], BF16)
        wdtp = ctx.enter_context(tc.tile_pool(name="wdtp", bufs=8))
        for hj in range(nht):
            wd_t = wstage.tile([128, dim], BF16, tag="wdt_build")
            nc.sync.dma_start(out=wd_t, in_=wdnq[:, hj * 128:(hj + 1) * 128],
                              transpose=True)
            nc.gpsimd.dma_start(out=wdnT_dram[hj * 128:(hj + 1) * 128, :],
                                in_=wd_t)

        # ---- x-prep for OWN tokens; chunked AG buffers -------------------
        xt_own = [dram.tile([dim, sb], BF16, tag=f"xto{c}", name=f"xto{c}")
                  for c in range(n_chunks)]
        s_own = [dram.tile([sb], F32, tag=f"so{c}", name=f"so{c}")
                 for c in range(n_chunks)]
        xt_all = [dram.tile([tp, dim, sb], BF16, tag=f"xta{c}",
                            name=f"xta{c}")
                  for c in range(n_chunks)]
        s_all = [dram.tile([tp, sb], F32, tag=f"sa{c}", name=f"sa{c}")
                 for c in range(n_chunks)]
        part_c = [dram.tile([tp * sb, dim], BF16, tag=f"pc{c}",
                            name=f"pc{c}")
                  for c in range(n_chunks)]
        red_c = [dram.tile([sb, dim], BF16, tag=f"rc{c}", name=f"rc{c}")
                 for c in range(n_chunks)]

        for c in range(n_chunks):
            t0 = c * sb
            xq_tiles = []
            for tt in range(tokt):
                row0 = t0 + tt * 128
                xt = xpool.tile([128, dim], F32, tag="xt")
                nc.gpsimd.dma_start(out=xt, in_=xs[row0:row0 + 128, :])
                xw = xpool.tile([128, dim], F32, tag="xw")
                ssq = small.tile([128, 1], F32, tag="ssq")
                nc.vector.tensor_tensor(out=xw, in0=xt, in1=xt, op=OP.mult)
                nc.vector.tensor_reduce(out=ssq, in_=xw, axis=AX.X, op=OP.add)
                am0 = small.tile([128, 1], F32, tag="am0")
                nc.vector.tensor_tensor(out=xw, in0=xt, in1=nw_b, op=OP.mult)
                nc.vector.tensor_reduce(out=am0, in_=xw, axis=AX.X, op=OP.max,
                                        apply_absolute_value=True)
                sig = small.tile([128, 1], F32, tag="sig")
                nc.scalar.activation(out=sig, in_=ssq, func=ACT.Sqrt,
                                     bias=eps_b, scale=1.0 / dim)
                rstd = small.tile([128, 1], F32, tag="rstd")
                nc.vector.reciprocal(out=rstd, in_=sig)
                gt = small.tile([128, 1], F32, tag="gt")
                nc.vector.tensor_scalar(out=gt, in0=am0, scalar1=rstd,
                                        scalar2=EPS_Q, op0=OP.mult, op1=OP.max)
                invg = small.tile([128, 1], F32, tag="invg")
                nc.vector.reciprocal(out=invg, in_=gt)
                rc = small.tile([128, 1], F32, tag="rc")
                nc.vector.tensor_scalar(out=rc, in0=invg, scalar1=rstd,
                                        scalar2=QB, op0=OP.mult, op1=OP.mult)
                nc.vector.tensor_scalar(out=xw, in0=xw, scalar1=rc, scalar2=R,
                                        op0=OP.mult, op1=OP.add)
                xq = xpool.tile([128, dim], BF16, tag="xq", bufs=tokt + 1)
                nc.vector.tensor_scalar(out=xq, in0=xw, scalar1=-R,
                                        scalar2=None, op0=OP.add)
                xq_tiles.append(xq)
                nc.gpsimd.dma_start(out=s_own[c][tt * 128:(tt + 1) * 128],
                                    in_=gt)
            for dj in range(ndb):
                pxp = psx.tile([128, sb], BF16, tag="xp")
                for tt in range(tokt):
                    nc.tensor.transpose(
                        pxp[:, tt * 128:(tt + 1) * 128],
                        xq_tiles[tt][:, dj * 128:(dj + 1) * 128], ident)
                xts = xpool.tile([128, sb], BF16, tag="xts", bufs=2)
                nc.vector.tensor_copy(out=xts, in_=pxp)
                nc.gpsimd.dma_start(
                    out=xt_own[c][dj * 128:(dj + 1) * 128, :], in_=xts)
            nc.gpsimd.collective_compute(
                "AllGather", OP.bypass, replica_groups=groups,
                ins=[xt_own[c][:]], outs=[xt_all[c][:]])
            nc.gpsimd.collective_compute(
                "AllGather", OP.bypass, replica_groups=groups,
                ins=[s_own[c][:]], outs=[s_all[c][:]])

        # ---- main compute: chunks x ranks --------------------------------
        for c in range(n_chunks):
            for rr in range(tp):
                # load this rank-chunk's Xt (one big DMA) + s_eff broadcast
                xt_sb = xtp.tile([128, ndb, sb], BF16, tag="xt_sb")
                nc.gpsimd.dma_start(
                    out=xt_sb,
                    in_=xt_all[c][rr].rearrange("(dj p) t -> p dj t", p=128))
                s_eff = xtp.tile([128, sb], F32, tag="seff")
                nc.gpsimd.dma_start(
                    out=s_eff,
                    in_=s_all[c][rr][None].to_broadcast((128, sb)))
                nc.vector.tensor_scalar(out=s_eff, in0=s_eff,
                                        scalar1=su127_b, scalar2=None,
                                        op0=OP.mult)

                # mm1: H^T tiles + scale/silu (scale in-place in PSUM)
                ht_tiles = []
                for hj in range(nht):
                    ph = ps1.tile([128, sb], F32, tag="mm1")
                    for dj in range(ndb):
                        nc.tensor.matmul(
                            ph, lhsT=wupT[dj][:, hj * 128:(hj + 1) * 128],
                            rhs=xt_sb[:, dj, :], start=(dj == 0),
                            stop=(dj == ndb - 1))
                    nc.vector.tensor_tensor(out=ph, in0=ph, in1=s_eff,
                                            op=OP.mult)
                    htt = htp.tile([128, sb], BF16, tag=f"ht{hj}",
                                   name=f"ht{hj}")
                    nc.scalar.activation(out=htt, in_=ph, func=ACT.Silu)
                    ht_tiles.append(htt)

                # mm2: partial out[tok, dout] over local h slice -> bf16
                for g in range(ng):
                    pos = [ps2.tile([128, doutg], F32, tag=f"mm2_{t}",
                                    name=f"mm2_{t}") for t in range(tokt)]
                    for hj in range(nht):
                        wdT = wdtp.tile([128, doutg], BF16, tag="wdT")
                        nc.scalar.dma_start(
                            out=wdT,
                            in_=wdnT_dram[hj * 128:(hj + 1) * 128,
                                          g * doutg:(g + 1) * doutg])
                        for tt in range(tokt):
                            nc.tensor.matmul(
                                pos[tt],
                                lhsT=ht_tiles[hj][:, tt * 128:(tt + 1) * 128],
                                rhs=wdT, start=(hj == 0),
                                stop=(hj == nht - 1))
                    for tt in range(tokt):
                        ob = opool.tile([128, doutg], BF16, tag="ob")
                        nc.vector.tensor_copy(out=ob, in_=pos[tt])
                        nc.gpsimd.dma_start(
                            out=part_c[c][rr * sb + tt * 128:
                                          rr * sb + (tt + 1) * 128,
                                          g * doutg:(g + 1) * doutg],
                            in_=ob)
            nc.gpsimd.collective_compute(
                "ReduceScatter", OP.add, replica_groups=groups,
                ins=[part_c[c][:]], outs=[red_c[c][:]])

        # ---- epilogue: own tokens: x + red * gamma_eff -------------------
        for c in range(n_chunks):
            for tt in range(tokt):
                row0 = c * sb + tt * 128
                rd = xpool.tile([128, dim], BF16, tag="xq", bufs=tokt + 1, name="rd")
                nc.gpsimd.dma_start(out=rd,
                                    in_=red_c[c][tt * 128:(tt + 1) * 128, :])
                o = xpool.tile([128, dim], F32, tag="xt")
                nc.vector.tensor_tensor(out=o, in0=rd, in1=ge, op=OP.mult)
                xr = xpool.tile([128, dim], F32, tag="xw")
                nc.gpsimd.dma_start(out=xr, in_=xs[row0:row0 + 128, :])
                nc.vector.tensor_tensor(out=o, in0=o, in1=xr, op=OP.add)
                nc.gpsimd.dma_start(out=ys[row0:row0 + 128, :], in_=o)


_PROGRAM_CACHE = {}


def _get_program(cfg):
    key = ("tp", cfg["dim"], cfg["hid"], cfg["B"], cfg["S"], cfg["sb"],
           cfg["n_chunks"])
    if key not in _PROGRAM_CACHE:
        _PROGRAM_CACHE[key] = build_program(cfg)
    return _PROGRAM_CACHE[key]


def make_in_maps(cfg, x, weight_up, weight_down, norm_weight, gamma):
    n_cores, tp = cfg["n_cores"], cfg["tp"]
    dp = n_cores // tp
    dim, hid = cfg["dim"], cfg["hid"]
    ntok = cfg["B"] * cfg["S"]
    grp_tok = ntok // dp
    own = grp_tok // tp

    x2 = np.ascontiguousarray(x.reshape(ntok, dim).astype(np.float32))
    wu = np.ascontiguousarray(weight_up.astype(np.float32))
    wd = np.ascontiguousarray(weight_down.astype(np.float32))
    nwv = np.ascontiguousarray(norm_weight.astype(np.float32))
    gmv = np.ascontiguousarray(gamma.astype(np.float32))
    usr = hid // n_cores
    dsr = dim // n_cores
    h_loc = hid // tp

    in_maps = []
    for core in range(n_cores):
        g, r = core // tp, core % tp
        row0 = g * grp_tok + r * own
        in_maps.append({
            "xs": x2[row0:row0 + own],
            "wup_tp": wu[r * h_loc:(r + 1) * h_loc],
            "wdn_tp": np.ascontiguousarray(wd[:, r * h_loc:(r + 1) * h_loc]),
            "nw": nwv,
            "gm": gmv,
            "wup_sl": wu[core * usr:(core + 1) * usr],
            "wdn_sl": wd[core * dsr:(core + 1) * dsr],
        })
    return in_maps


def run(cfg, x, weight_up, weight_down, norm_weight, gamma, **run_kwargs):
    n_cores, tp = cfg["n_cores"], cfg["tp"]
    dp = n_cores // tp
    dim = cfg["dim"]
    ntok = cfg["B"] * cfg["S"]
    grp_tok = ntok // dp
    own = grp_tok // tp

    nc = _get_program(cfg)
    in_maps = make_in_maps(cfg, x, weight_up, weight_down, norm_weight, gamma)
    res = run_bass_kernel_spmd(nc, in_maps, core_ids=list(range(n_cores)),
                               **run_kwargs)
    out = np.concatenate([res.results[c]["ys"] for c in range(n_cores)],
                         axis=0)
    return out.reshape(cfg["B"], cfg["S"], dim), res


full_cfg = tp_full_cfg


def kernel(x, weight_up, weight_down, norm_weight, gamma):
    out, _ = run(tp_full_cfg(), x, weight_up, weight_down, norm_weight, gamma)
    return out.astype(np.float32)


if __name__ == "__main__":
    nc = build_program(tp_full_cfg())
    print("build OK")
